# revision 63
# baseline (speedup 1.0000x reference)
"""Dinov3 ViT attention kernel for Trainium2 (8 NeuronCores, data-parallel over batch).

Per core: 2 batch items. hidden_states [2*1029, 1024] in, out [2*1029, 1024] f32.

Key structure (v2):
  - Q/K/V projections run as fp8e4 DoubleRow matmuls (2 k-tiles per
    instruction, 0.5 cyc/col): host supplies hi = fp8(x) and lo = fp8(16*(x-hi))
    splits of hidden_states and wq/wk/wv; terms hi@hi (scale 1) and
    hi@lo + lo@hi (scale 1/16) accumulate in two PSUM groups, merged per
    chunk as dst = A + bf16(B*1/16) (tensor_scalar through SBUF, since DVE
    ops may read only one PSUM operand). O-projection stays bf16.
  - Q/K head dims are pair-interleaved (d, d+32) by permuting weight columns
    on the host, so RoPE's rotate-half is an adjacent-lane swap done with two
    partition-strided SBUF->SBUF DMAs; cos/sin tables arrive pre-transposed,
    pre-permuted and pre-signed. Q bias is added in place on the Pool engine;
    V bias is folded into bo' = bo + bv @ wo on the host.
  - Attention value matmul runs transposed: stationary = exp(S^T) query-tile
    [keys,128], moving = V [keys,65] -> PSUM [query,65] token-major, 65 cols
    per (head, qtile, keytile) instead of 128 q-cols per key tile. The softmax
    denominator lands per-partition, so normalize is a [128,2] reciprocal +
    one free-dim-broadcast multiply per (head, 2 qtiles). AOT (feature-major,
    needed by the O-projection) is rebuilt with PE transposes + DVE copies
    (dma_start_transpose corrupts data with offset APs here).
  - The 5 tail keys (1024:1029) are handled per head-pair: S^T rows 0:5
    (even head) and 32:37 (odd head) of one PSUM tile, a single exp, and one
    extra link in each AV accumulation chain (odd head V-tail is DMA-copied
    to partitions 32:37 once per item).
"""
import sys
import time

sys.path.insert(0, "/opt/trn_rl_repo")

import ml_dtypes
import numpy as np

import concourse.bacc as bacc
import concourse.mybir as mybir
import concourse.tile as tile

f32 = mybir.dt.float32
bf16 = mybir.dt.bfloat16
fp8 = mybir.dt.float8e4
FP = mybir.ActivationFunctionType
ADD = mybir.AluOpType.add
MUL = mybir.AluOpType.mult
DR = mybir.MatmulPerfMode.DoubleRow

H = 1024
NH = 16
HD = 64
T = 1029
NPREF = 5
PATCH = 1024
B = 16
NCORES = 8
BPC = B // NCORES          # batch items per core
KO = H // 128              # 8 feature k-tiles
KP = KO // 2               # 4 k-tile pairs for DoubleRow
TOK = BPC * T              # tokens per core (2058)
SCALE = 1.0 / float(np.sqrt(HD))
LO = 1.0 / 16.0            # lo-term scale

NJT = 8                    # full 128-key tiles (keys 0..1024)
NQT = 8                    # full 128-query tiles (queries 0..1024)
QTAIL = (1024, T - 1024)   # 5 queries -> batched-exp path
# fp8 DR moving operand is 2x chunk wide; cap 512 -> chunks of 256 tokens
DRCH = [(0, 256), (256, 256), (512, 256), (768, 256), (1024, 5)]
NCH = [(0, 256), (256, 256), (512, 256), (768, 256)]   # V out-dim chunks
TOK_TILES = [(i * 128, min(128, T - i * 128)) for i in range((T + 127) // 128)]
OCHUNKS = [(0, 512), (512, 512)]


def build():
    nc = bacc.Bacc(None, target_bir_lowering=False)
    hsh = nc.dram_tensor("hsh", [H, TOK], fp8, kind="ExternalInput")
    hsl = nc.dram_tensor("hsl", [H, TOK], fp8, kind="ExternalInput")
    cos_d = nc.dram_tensor("cos_t", [HD, PATCH], bf16, kind="ExternalInput")
    sin_d = nc.dram_tensor("sin_t", [HD, PATCH], bf16, kind="ExternalInput")
    w_d = {}
    for wn in ("wq", "wk", "wv"):
        w_d[wn + "h"] = nc.dram_tensor(wn + "h", [H, H], fp8,
                                       kind="ExternalInput")
        w_d[wn + "l"] = nc.dram_tensor(wn + "l", [H, H], fp8,
                                       kind="ExternalInput")
    wo_d = nc.dram_tensor("wo", [H, H], bf16, kind="ExternalInput")
    bq_d = nc.dram_tensor("bq", [H], f32, kind="ExternalInput")
    bo_d = nc.dram_tensor("bo2", [H], bf16, kind="ExternalInput")
    ident_d = nc.dram_tensor("ident", [128, 128], bf16, kind="ExternalInput")
    out_d = nc.dram_tensor("out", [TOK, H], f32, kind="ExternalOutput")

    with tile.TileContext(nc) as tc:
        with (
            tc.tile_pool(name="const", bufs=1) as cpool,
            tc.tile_pool(name="item", bufs=1) as ipool,
            tc.tile_pool(name="ao", bufs=2) as aopool,
            tc.tile_pool(name="aop", bufs=2) as aoppool,
            tc.tile_pool(name="rope", bufs=3) as rpool,
            tc.tile_pool(name="attn", bufs=10) as apool,
            tc.tile_pool(name="ypool", bufs=2) as ypool,
            tc.tile_pool(name="misc", bufs=2) as mpool,
            tc.tile_pool(name="ps_s", bufs=2, space="PSUM") as ps_s,
            tc.tile_pool(name="ps_po", bufs=2, space="PSUM") as ps_po,
            tc.tile_pool(name="ps_w", bufs=2, space="PSUM") as ps_w,
        ):
            hsh_r = hsh.rearrange("(o p) t -> p o t", p=128)
            hsl_r = hsl.rearrange("(o p) t -> p o t", p=128)

            def emit_xprep_full(bi, XTh, XTl):
                # split by token halves so early tiles unblock sooner
                b0 = bi * T
                for (t0, t1) in ((0, 512), (512, T)):
                    nc.sync.dma_start(XTh[:, :, t0:t1],
                                      hsh_r[:, :, b0 + t0: b0 + t1])
                    nc.sync.dma_start(XTl[:, :, t0:t1],
                                      hsl_r[:, :, b0 + t0: b0 + t1])

            XTh0 = ipool.tile([128, KO, 1056], fp8, tag="XTh", name="XTh_0")
            XTl0 = ipool.tile([128, KO, 1056], fp8, tag="XTl", name="XTl_0")
            emit_xprep_full(0, XTh0, XTl0)

            # weights first (V needed earliest), then tables, wo/bo last
            wb = {}
            for wn in ("wvh", "wvl", "wqh", "wql", "wkh", "wkl"):
                wb[wn] = cpool.tile([128, KO, H], fp8, tag=f"wb_{wn}",
                                    name=f"wb_{wn}")
                nc.sync.dma_start(
                    wb[wn][:], w_d[wn].rearrange("(o p) n -> p o n", p=128))

            bq_sb = cpool.tile([128, KO], f32)
            nc.sync.dma_start(bq_sb[:], bq_d.rearrange("(o p) -> p o", p=128))
            # cos/sin arrive pre-transposed/permuted/signed (bf16 [64, 1024])
            cosT2 = cpool.tile([128, PATCH], bf16)
            sinT2 = cpool.tile([128, PATCH], bf16)
            nc.sync.dma_start(cosT2[0:64, :], cos_d[:, :])
            nc.sync.dma_start(cosT2[64:128, :], cos_d[:, :])
            nc.sync.dma_start(sinT2[0:64, :], sin_d[:, :])
            nc.sync.dma_start(sinT2[64:128, :], sin_d[:, :])
            identb = cpool.tile([128, 128], bf16)
            nc.sync.dma_start(identb[:], ident_d[:])
            wbo = cpool.tile([128, KO, H], bf16, tag="wb_wo", name="wb_wo")
            nc.sync.dma_start(wbo[:], wo_d.rearrange("(o p) n -> p o n", p=128))
            bo_bc = cpool.tile([128, H], bf16)
            nc.sync.dma_start(bo_bc[:], bo_d[None, :].to_broadcast((128, H)))

            # ---------------- per batch item ----------------
            def make_item(bi, XTh, XTl):
                tok0 = bi * T
                QT = ipool.tile([128, KO, T], bf16, tag="QT", name=f"QT_{bi}")
                KT = ipool.tile([128, KO, T], bf16, tag="KT", name=f"KT_{bi}")
                Vst = ipool.tile([128, NJT + 1, NH, HD + 1], bf16, tag="Vst",
                                 name=f"Vst_{bi}")
                Vt32 = ipool.tile([128, KO, HD + 1], bf16, tag="Vt32",
                                  name=f"Vt32_{bi}")
                AOT = aopool.tile([128, KO, T], bf16, tag="AOT",
                                  name=f"AOT_{bi}")

                def emit_vinit():
                    nc.vector.memset(Vst[:, :, :, HD:HD + 1], 1.0)

                def emit_vproj(ti, ci):
                    # out token-major [tw, 4 heads x 64]; DR over k-tile pairs
                    n0, nw = NCH[ci]
                    t0, tw = TOK_TILES[ti]
                    mw = 32 if tw < 32 else tw   # ISA min for DR stationary
                    pm = ps_w.tile([128, 2, 4, 64], f32, tag="ps_w",
                                   name=f"pmv_{bi}_{ci}_{ti}")
                    for kp in range(KP):
                        nc.tensor.matmul(
                            pm[:mw, 0, :, :],
                            XTh[:, 2 * kp:2 * kp + 2, t0:t0 + mw],
                            wb["wvh"][:, 2 * kp:2 * kp + 2, n0:n0 + nw],
                            start=(kp == 0), stop=(kp == KP - 1), perf_mode=DR)
                    for kp in range(KP):
                        nc.tensor.matmul(
                            pm[:mw, 1, :, :],
                            XTh[:, 2 * kp:2 * kp + 2, t0:t0 + mw],
                            wb["wvl"][:, 2 * kp:2 * kp + 2, n0:n0 + nw],
                            start=(kp == 0), stop=False, perf_mode=DR)
                    for kp in range(KP):
                        nc.tensor.matmul(
                            pm[:mw, 1, :, :],
                            XTl[:, 2 * kp:2 * kp + 2, t0:t0 + mw],
                            wb["wvh"][:, 2 * kp:2 * kp + 2, n0:n0 + nw],
                            start=False, stop=(kp == KP - 1), perf_mode=DR)
                    bsb = mpool.tile([128, 4, 64], bf16, tag="bsb",
                                     bufs=3, name=f"bsbv_{bi}_{ci}_{ti}")
                    nc.vector.tensor_scalar_mul(bsb[:tw], pm[:tw, 1, :, :], LO)
                    nc.vector.tensor_tensor(
                        Vst[:tw, ti, 4 * ci:4 * ci + 4, 0:HD],
                        pm[:tw, 0, :, :], bsb[:tw], ADD)

                def emit_vtail32():
                    # odd heads' V tail rows -> partitions 32:37
                    for kq in range(KO):
                        nc.sync.dma_start(
                            Vt32[32:37, kq, :],
                            Vst[0:5, NJT, 2 * kq + 1, :])

                def emit_qkproj(mo, which, ci):
                    dst, wh, wl = ((QT, "wqh", "wql"),
                                   (KT, "wkh", "wkl"))[which]
                    q0, qw = DRCH[ci]
                    mw = 32 if qw < 32 else qw
                    pm = ps_w.tile([128, 2, 256], f32, tag="ps_w",
                                   name=f"pm_{bi}_{which}_{mo}_{q0}")
                    for kp in range(KP):
                        nc.tensor.matmul(
                            pm[:, 0, :mw],
                            wb[wh][:, 2 * kp:2 * kp + 2, mo * 128:(mo + 1) * 128],
                            XTh[:, 2 * kp:2 * kp + 2, q0:q0 + mw],
                            start=(kp == 0), stop=(kp == KP - 1), perf_mode=DR)
                    for kp in range(KP):
                        nc.tensor.matmul(
                            pm[:, 1, :mw],
                            wb[wl][:, 2 * kp:2 * kp + 2, mo * 128:(mo + 1) * 128],
                            XTh[:, 2 * kp:2 * kp + 2, q0:q0 + mw],
                            start=(kp == 0), stop=False, perf_mode=DR)
                    for kp in range(KP):
                        nc.tensor.matmul(
                            pm[:, 1, :mw],
                            wb[wh][:, 2 * kp:2 * kp + 2, mo * 128:(mo + 1) * 128],
                            XTl[:, 2 * kp:2 * kp + 2, q0:q0 + mw],
                            start=False, stop=(kp == KP - 1), perf_mode=DR)
                    bsb = mpool.tile([128, 256], bf16, tag="bsb", bufs=4,
                                     name=f"bsb_{bi}_{which}_{mo}_{q0}")
                    nc.vector.tensor_scalar_mul(bsb[:, :qw], pm[:, 1, :qw], LO)
                    nc.vector.tensor_tensor(
                        dst[:, mo, q0:q0 + qw], pm[:, 0, :qw], bsb[:, :qw],
                        ADD)

                def emit_rope(mo, which):
                    # out = qb*cos + swap(qb)*sin_eff; qb = q (+bq for Q, done
                    # in-place on the Pool engine); swap = adjacent-pair lane
                    # swap via a partition-strided SBUF->SBUF DMA
                    tgt = (QT, KT)[which]
                    if which == 0:
                        nc.gpsimd.tensor_tensor(
                            tgt[:, mo, :], tgt[:, mo, :],
                            bq_sb[:, mo:mo + 1].to_broadcast((128, T)), ADD)
                    src = tgt[:, mo, NPREF:T]
                    qs = rpool.tile([128, PATCH], bf16, tag="rt", name="qs")
                    sv = src.rearrange("(a two) f -> a two f", two=2)
                    qv = qs[:, :].rearrange("(a two) f -> a two f", two=2)
                    nc.sync.dma_start(qv[:, 0, :], sv[:, 1, :])
                    nc.sync.dma_start(qv[:, 1, :], sv[:, 0, :])
                    t1 = rpool.tile([128, PATCH], bf16, tag="rt", name="t1")
                    nc.vector.tensor_tensor(t1[:], src, cosT2[:], MUL)
                    t2 = rpool.tile([128, PATCH], bf16, tag="rt", name="t2")
                    nc.vector.tensor_tensor(t2[:], qs[:], sinT2[:], MUL)
                    nc.vector.tensor_tensor(src, t1[:], t2[:], ADD)

                def emit_st(h, ji, pump=None):
                    # S^T tile [128 keys, 1024 queries] + exp -> es
                    kq = h // 2
                    ph = (h % 2) * 64
                    j0 = ji * 128
                    pss = ps_s.tile([128, 1024], f32, tag="ps_s")
                    for q0 in (0, 512):
                        nc.tensor.matmul(
                            pss[:, q0:q0 + 512],
                            KT[ph:ph + 64, kq, j0:j0 + 128],
                            QT[ph:ph + 64, kq, q0:q0 + 512],
                            start=True, stop=True)
                    es = apool.tile([128, 1024], bf16, tag="expS",
                                    name=f"es_{bi}_{h}_{ji}")
                    nc.scalar.activation(es[:, :], pss[:, :], FP.Exp,
                                         scale=SCALE)
                    return es

                def emit_st_tail(mo, pump=None):
                    # pair tail keys: even head rows 0:5, odd rows 32:37
                    kq = mo
                    pss = ps_s.tile([128, 1024], f32, tag="ps_s",
                                    name=f"pstl_{bi}_{mo}")
                    for off, ph in ((0, 0), (32, 64)):
                        for q0 in (0, 512):
                            nc.tensor.matmul(
                                pss[off:off + 5, q0:q0 + 512],
                                KT[ph:ph + 64, kq, 1024:1029],
                                QT[ph:ph + 64, kq, q0:q0 + 512],
                                start=True, stop=True)
                    est = apool.tile([128, 1024], bf16, tag="expS",
                                     name=f"est_{bi}_{mo}")
                    nc.scalar.activation(est[0:37, :], pss[0:37, :], FP.Exp,
                                         scale=SCALE)
                    return est

                def emit_avt(h, es_list, est, AOp, pump=None):
                    # transposed AV: per 2 qtiles, chains for 8+1 key tiles
                    ph32 = (h % 2) * 32
                    kq = h // 2
                    for qp in range(NQT // 2):
                        po = ps_po.tile([128, 2, HD + 1], f32, tag="po",
                                        name=f"po_{bi}_{h}_{qp}")
                        for sub in range(2):
                            qt = 2 * qp + sub
                            q0 = qt * 128
                            if pump is not None:
                                pump()
                                pump()
                                pump()
                            for ji in range(NJT):
                                nc.tensor.matmul(
                                    po[:, sub, :],
                                    es_list[ji][:, q0:q0 + 128],
                                    Vst[:, ji, h, :],
                                    start=(ji == 0), stop=False)
                            vt = Vst[0:5, NJT, h, :] if h % 2 == 0 \
                                else Vt32[32:37, kq, :]
                            nc.tensor.matmul(
                                po[:, sub, :],
                                est[ph32:ph32 + 5, q0:q0 + 128],
                                vt, start=False, stop=True)
                        rc = mpool.tile([128, 2, 1], f32, tag="rc")
                        nc.vector.reciprocal(rc[:, :, :], po[:, :, HD:HD + 1])
                        nc.vector.tensor_tensor(
                            AOp[:, 2 * qp:2 * qp + 2, h % 2, :],
                            po[:, :, 0:HD],
                            rc[:, :, :].to_broadcast((128, 2, HD)), MUL)

                def emit_ao_transpose(mo, AOp):
                    for qg in range(NQT // 2):
                        pt = ps_po.tile([128, 2, 128], bf16, tag="po",
                                        name=f"pt_{bi}_{mo}_{qg}")
                        for k in range(2):
                            qt = 2 * qg + k
                            nc.tensor.transpose(
                                pt[:, k, :], AOp[:, qt, :, :], identb[:])
                        nc.vector.tensor_copy(
                            AOT[:, mo, qg * 256:(qg + 1) * 256],
                            pt[:, :, :])

                def emit_tail():
                    # 5-query tail for all 16 heads (feature-major, as v1)
                    qt0, qtw = QTAIL
                    pst = ps_s.tile([128, 1024], f32, tag="ps_s",
                                    name=f"pst_{bi}")

                    def tcol(h):
                        return (h * qtw * 9 if h <= 10
                                else 512 + (h - 11) * qtw * 9)

                    for h in range(NH):
                        ph = (h % 2) * 64
                        kq = h // 2
                        for ji, (j0, jw) in enumerate(TOK_TILES):
                            nc.tensor.matmul(
                                pst[:jw,
                                    tcol(h) + ji * qtw: tcol(h) + (ji + 1) * qtw],
                                KT[ph:ph + 64, kq, j0:j0 + jw],
                                QT[ph:ph + 64, kq, qt0:qt0 + qtw],
                                start=True, stop=True)
                    est = apool.tile([128, 1024], bf16, tag="expS",
                                     name=f"estq_{bi}")
                    nc.scalar.activation(est[:, 0:495], pst[:, 0:495],
                                         FP.Exp, scale=SCALE)
                    nc.scalar.activation(est[:, 512:737], pst[:, 512:737],
                                         FP.Exp, scale=SCALE)
                    pot2 = ps_po.tile([128, 130], f32, tag="po",
                                      name=f"pot_{bi}")
                    for h in range(NH):
                        for ji, (j0, jw) in enumerate(TOK_TILES):
                            nc.tensor.matmul(
                                pot2[:HD + 1, h * qtw:(h + 1) * qtw],
                                Vst[:jw, ji, h, :],
                                est[0:jw,
                                    tcol(h) + ji * qtw: tcol(h) + (ji + 1) * qtw],
                                start=(ji == 0), stop=(ji == 8))
                    rc = mpool.tile([1, 128], f32, tag="rct", bufs=1)
                    nc.vector.reciprocal(rc[0:1, :NH * qtw],
                                         pot2[HD:HD + 1, :NH * qtw])
                    rb = mpool.tile([64, 128], f32, tag="rbt", bufs=1)
                    nc.gpsimd.partition_broadcast(rb[:, :NH * qtw],
                                                  rc[0:1, :NH * qtw])
                    for h in range(NH):
                        nc.vector.tensor_tensor(
                            AOT[(h % 2) * 64:(h % 2) * 64 + 64, h // 2,
                                qt0:qt0 + qtw],
                            pot2[0:HD, h * qtw:(h + 1) * qtw],
                            rb[:, h * qtw:(h + 1) * qtw], MUL)

                def emit_outproj_g(ti, nci):
                    t0, tw = TOK_TILES[ti]
                    n0, nw = OCHUNKS[nci]
                    pm = ps_w.tile([128, 512], f32, tag="ps_w",
                                   name=f"pmo_{bi}_{ti}_{n0}")
                    for ko in range(KO):
                        nc.tensor.matmul(
                            pm[:tw, :nw],
                            AOT[:, ko, t0:t0 + tw],
                            wbo[:, ko, n0:n0 + nw],
                            start=(ko == 0), stop=(ko == KO - 1))
                    y = ypool.tile([128, 512], f32, tag="y")
                    nc.vector.tensor_tensor(y[:tw, :nw], pm[:tw, :nw],
                                            bo_bc[:tw, n0:n0 + nw], ADD)
                    nc.sync.dma_start(
                        out_d[tok0 + t0: tok0 + t0 + tw, n0:n0 + nw],
                        y[:tw, :nw])

                def emit_outproj(skip=()):
                    for ti in range(len(TOK_TILES)):
                        for nci in range(len(OCHUNKS)):
                            if (ti, nci) not in skip:
                                emit_outproj_g(ti, nci)

                def emit_pair(mo, pump):
                    he, ho = 2 * mo, 2 * mo + 1
                    es_e = [emit_st(he, ji, pump) for ji in range(NJT)]
                    est = emit_st_tail(mo, pump)
                    es_o = [emit_st(ho, ji, pump) for ji in range(NJT)]
                    AOp = aoppool.tile([128, NQT, 2, HD], bf16, tag="AOp",
                                       name=f"AOp_{bi}_{mo}")
                    emit_avt(he, es_e, est, AOp, pump)
                    emit_avt(ho, es_o, est, AOp, pump)
                    emit_ao_transpose(mo, AOp)

                def emit_blocks(extra=None):
                    fills = []

                    def pump():
                        if fills:
                            fills.pop(0)()

                    for mo in range(KO):
                        if mo in (0, 2, 4):
                            ci = 1 + mo // 2
                            fills.extend(
                                (lambda ti=ti, ci=ci: emit_vproj(ti, ci))
                                for ti in range(NJT))
                        if mo < KO - 1:
                            fills.extend(
                                (lambda mo=mo, which=which, ci=ci:
                                 emit_qkproj(mo + 1, which, ci))
                                for which in range(2)
                                for ci in range(len(DRCH)))
                            fills.append(lambda mo=mo: emit_rope(mo + 1, 0))
                            fills.append(lambda mo=mo: emit_rope(mo + 1, 1))
                        if extra and mo in extra:
                            fills.extend(extra[mo])
                        emit_pair(mo, pump)
                    while fills:
                        fills.pop(0)()

                def emit_head(fills=None):
                    fills = list(fills or [])

                    def pump():
                        if fills:
                            fills.pop(0)()

                    emit_vinit()
                    for ti in range(4):
                        emit_vproj(ti, 0)
                    for ci in range(4):
                        emit_vproj(8, ci)
                    emit_vtail32()
                    for ti in range(4, NJT):
                        emit_vproj(ti, 0)
                        pump()
                    for which in range(2):
                        for ci in range(len(DRCH)):
                            emit_qkproj(0, which, ci)
                            pump()
                    emit_rope(0, 0)
                    emit_rope(0, 1)
                    while fills:
                        fills.pop(0)()

                return {
                    "head": emit_head, "blocks": emit_blocks,
                    "tail": emit_tail, "outproj": emit_outproj,
                    "outproj_g": emit_outproj_g,
                }

            it0 = make_item(0, XTh0, XTl0)
            it0["head"]()
            XTh1 = ipool.tile([128, KO, 1056], fp8, tag="XTh", name="XTh_1")
            XTl1 = ipool.tile([128, KO, 1056], fp8, tag="XTl", name="XTl_1")
            it0["blocks"](extra={4: [lambda: emit_xprep_full(1, XTh1, XTl1)]})
            it0["tail"]()
            it1 = make_item(1, XTh1, XTl1)
            it1["head"]()              # runs during item0 out-proj
            defer = [(ti, nci) for ti in range(3, len(TOK_TILES))
                     for nci in range(len(OCHUNKS))]
            it0["outproj"](skip=defer)
            dthunks = [(lambda ti=ti, nci=nci: it0["outproj_g"](ti, nci))
                       for (ti, nci) in defer]
            it1["blocks"](extra={5: dthunks[0:4], 6: dthunks[4:8],
                                 7: dthunks[8:12]})
            it1["tail"]()
            it1["outproj"]()

    nc.compile()
    return nc


_NC_CACHE = []
_LAST_RESULT = []


def _prep_inputs(hidden_states, cos, sin, wq, bq, wk, wv, bv, wo, bo):
    e4 = ml_dtypes.float8_e4m3

    def f8(x):
        return np.clip(np.asarray(x, np.float32), -240, 240).astype(e4)

    def split8(x):
        x = np.asarray(x, np.float32)
        hi = f8(x)
        lo = f8((x - hi.astype(np.float32)) * 16.0)
        return hi, lo

    # pair-interleave permutation of head dims for Q/K (rotate-half partners
    # become adjacent lanes)
    perm64 = np.empty(HD, np.int64)
    perm64[0::2] = np.arange(32)
    perm64[1::2] = np.arange(32) + 32
    permH = (np.repeat(np.arange(NH) * HD, HD) + np.tile(perm64, NH))

    wq_p = np.asarray(wq, np.float32)[:, permH]
    wk_p = np.asarray(wk, np.float32)[:, permH]
    bq_p = np.asarray(bq, np.float32)[permH]
    cos_t = np.ascontiguousarray(
        np.asarray(cos, np.float32)[:, perm64].T.astype(ml_dtypes.bfloat16))
    sin_e = np.asarray(sin, np.float32)[:, perm64].copy()
    sin_e[:, 0::2] *= -1.0     # sign for the even (dest) lanes
    sin_t = np.ascontiguousarray(sin_e.T.astype(ml_dtypes.bfloat16))

    bo2 = (np.asarray(bo, np.float32)
           + np.asarray(bv, np.float32) @ np.asarray(wo, np.float32))

    wqh, wql = split8(wq_p)
    wkh, wkl = split8(wk_p)
    wvh, wvl = split8(np.asarray(wv, np.float32))

    hs2 = np.asarray(hidden_states, np.float32).reshape(B * T, H)
    shared = {
        "ident": np.eye(128, dtype=ml_dtypes.bfloat16),
        "cos_t": cos_t, "sin_t": sin_t,
        "wqh": np.ascontiguousarray(wqh), "wql": np.ascontiguousarray(wql),
        "wkh": np.ascontiguousarray(wkh), "wkl": np.ascontiguousarray(wkl),
        "wvh": np.ascontiguousarray(wvh), "wvl": np.ascontiguousarray(wvl),
        "wo": np.ascontiguousarray(
            np.asarray(wo, np.float32).astype(ml_dtypes.bfloat16)),
        "bq": np.ascontiguousarray(bq_p),
        "bo2": np.ascontiguousarray(bo2.astype(ml_dtypes.bfloat16)),
    }
    in_maps = []
    for c in range(NCORES):
        sl = hs2[c * TOK:(c + 1) * TOK].T   # [H, TOK]
        hh, hl = split8(sl)
        m = dict(shared)
        m["hsh"] = np.ascontiguousarray(hh)
        m["hsl"] = np.ascontiguousarray(hl)
        in_maps.append(m)
    return in_maps


def kernel(hidden_states, cos, sin, wq, bq, wk, wv, bv, wo, bo):
    from concourse.bass_utils import run_bass_kernel_spmd

    in_maps = _prep_inputs(hidden_states, cos, sin, wq, bq, wk, wv, bv,
                           wo, bo)
    if not _NC_CACHE:
        _NC_CACHE.append(build())
    nc = _NC_CACHE[0]

    try:
        res = run_bass_kernel_spmd(nc, in_maps, core_ids=list(range(NCORES)))
    except Exception:
        # transient NRT device errors have been observed on this fabric;
        # one retry usually succeeds
        time.sleep(2.0)
        res = run_bass_kernel_spmd(nc, in_maps, core_ids=list(range(NCORES)))
    _LAST_RESULT.clear()
    _LAST_RESULT.append(res)
    out = np.concatenate(
        [r["out"].reshape(BPC, T, H) for r in res.results], axis=0)
    return out
